# revision 1
# baseline (speedup 1.0000x reference)
"""DigitCaps (capsule routing) Trainium2 kernel.

Self-contained: hardcodes shapes for
  x: [256, 32, 8, 6, 6] f32, W: [1, 10, 1152, 16, 8] f32 -> v: [256, 10, 16] f32

Sharding: pure data parallelism over batch, 32 batch items per core on 8 cores.

Per-core on-chip layout: partition p = (i16, b8)  (16 in-capsule offsets x 8
batch items of the current octet group), free = (ic=72 chunks, h=10, w=16).
u = W@x is computed by block-diagonal packed matmuls (K=128 fully used):
lhsT is a host-prebuilt block-diagonal x tile [(i16,s8), (i16',b8)], rhs is
repacked W [(i16,s8), (h,w)] per chunk.  Routing reductions over in-capsules
run on TensorE via a static delta-selection matrix accumulated in PSUM over
the 72 chunks; softmax is kept unnormalized (1/d folded into squash); the
agreement reduce over w runs on DVE (mul + reduce-X in chunks); v is
broadcast across partitions with a second delta-matmul.  Iteration 3's
agreement update is dead code and skipped.
"""

import numpy as np

# ---- problem constants (hardcoded) ----
B_FULL = 256
N_CORES = 8
B_CORE = B_FULL // N_CORES          # 32
NGRP = 4                            # octet groups per core
B8 = 8                              # batch per group
H = 10
WD = 16
HW = H * WD                         # 160
S = 8
NI = 1152
I16 = 16
IC = NI // I16                      # 72
ICC = 9                             # ic per W-stream / mul chunk
NQ = IC // ICC                      # 8 chunks
CPY = 3                             # ic per psum copy tile
P = 128

_CACHE = {}


def _build_program(debug: bool):
    import concourse.bacc as bacc
    import concourse.bass as bass
    import concourse.tile as tile
    from concourse import mybir

    f32 = mybir.dt.float32
    AX = mybir.AxisListType

    nc = bacc.Bacc(
        "TRN2", target_bir_lowering=False, debug=debug, enable_asserts=False
    )

    xd_d = nc.dram_tensor("xdiag", [NGRP, P, IC * P], f32, kind="ExternalInput")
    w_d = nc.dram_tensor("wpack", [IC, P, HW], f32, kind="ExternalInput")
    sd_d = nc.dram_tensor("sdelta", [P, B8], f32, kind="ExternalInput")
    sr_d = nc.dram_tensor("srepl", [B8, P], f32, kind="ExternalInput")
    out_d = nc.dram_tensor("vout", [B_CORE, HW], f32, kind="ExternalOutput")

    with tile.TileContext(nc) as tc:
        with (
            tc.tile_pool(name="const", bufs=1) as const_pool,
            tc.tile_pool(name="xd", bufs=1) as xd_pool,
            tc.tile_pool(name="u", bufs=2) as u_pool,
            tc.tile_pool(name="wq", bufs=2) as w_pool,
            tc.tile_pool(name="scr", bufs=2) as scr_pool,
            tc.tile_pool(name="logits", bufs=2) as log_pool,
            tc.tile_pool(name="cexp", bufs=2) as c_pool,
            tc.tile_pool(name="small", bufs=2) as small_pool,
            tc.tile_pool(name="psum_u", bufs=2, space="PSUM") as psum_u,
            tc.tile_pool(name="psum_r", bufs=1, space="PSUM") as psum_r,
        ):
            sdelta = const_pool.tile([P, B8], f32, tag="sdelta")
            nc.sync.dma_start(sdelta[:], sd_d[:])
            srepl = const_pool.tile([B8, P], f32, tag="srepl")
            nc.sync.dma_start(srepl[:], sr_d[:])

            for g in range(NGRP):
                # ---- load block-diag x and compute u ----
                xd = xd_pool.tile([P, IC, P], f32, tag="xd")
                nc.sync.dma_start(xd[:], xd_d[g].rearrange("p (ic m) -> p ic m", ic=IC))

                u = u_pool.tile([P, IC, H, WD], f32, tag="u")
                for q in range(NQ):
                    wq = w_pool.tile([P, ICC, HW], f32, tag="wq")
                    nc.sync.dma_start(
                        wq[:],
                        w_d[q * ICC : (q + 1) * ICC].rearrange("ic p f -> p ic f"),
                    )
                    for j in range(0, ICC, CPY):
                        ps = psum_u.tile([P, CPY, HW], f32, tag="ups")
                        for t in range(CPY):
                            nc.tensor.matmul(
                                ps[:, t, :],
                                xd[:, q * ICC + j + t, :],
                                wq[:, j + t, :],
                                start=True,
                                stop=True,
                            )
                        nc.scalar.copy(
                            u[:, q * ICC + j : q * ICC + j + CPY].rearrange(
                                "p a h w -> p a (h w)"
                            ),
                            ps[:],
                        )

                # ---- routing ----
                logits = log_pool.tile([P, IC, H], f32, tag="logits")
                for it in range(3):
                    dinv = None
                    cexp = None
                    if it > 0:
                        cexp = c_pool.tile([P, IC, H], f32, tag="cexp")
                        nc.scalar.activation(
                            cexp[:], logits[:], mybir.ActivationFunctionType.Exp
                        )
                        dps = psum_r.tile([B8, H], f32, tag="dps")
                        for ic in range(IC):
                            nc.tensor.matmul(
                                dps[:],
                                sdelta[:],
                                cexp[:, ic, :],
                                start=(ic == 0),
                                stop=(ic == IC - 1),
                            )
                        dinv = small_pool.tile([B8, H], f32, tag="dinv")
                        nc.vector.reciprocal(dinv[:], dps[:])

                    # s_un = sum_i c_un * u   (PE delta-reduce, PSUM-accumulated)
                    sps = psum_r.tile([B8, HW], f32, tag="sps")
                    if it == 0:
                        for ic in range(IC):
                            nc.tensor.matmul(
                                sps[:],
                                sdelta[:],
                                u[:, ic].rearrange("p h w -> p (h w)"),
                                start=(ic == 0),
                                stop=(ic == IC - 1),
                            )
                    else:
                        for q in range(NQ):
                            pr = scr_pool.tile([P, ICC, H, WD], f32, tag="pr")
                            cb = (
                                cexp[:, q * ICC : (q + 1) * ICC]
                                .unsqueeze(3)
                                .to_broadcast([P, ICC, H, WD])
                            )
                            nc.vector.tensor_mul(
                                pr[:], u[:, q * ICC : (q + 1) * ICC], cb
                            )
                            for t in range(ICC):
                                ic = q * ICC + t
                                nc.tensor.matmul(
                                    sps[:],
                                    sdelta[:],
                                    pr[:, t].rearrange("p h w -> p (h w)"),
                                    start=(ic == 0),
                                    stop=(ic == IC - 1),
                                )

                    # ---- squash: v = s * sqrt(sq) / (1 + sq), s = s_un / d ----
                    s = small_pool.tile([B8, H, WD], f32, tag="s")
                    spsv = sps[:].rearrange("b (h w) -> b h w", h=H)
                    if it == 0:
                        nc.vector.tensor_scalar_mul(s[:], spsv, 1.0 / NI)
                    else:
                        db = dinv[:].unsqueeze(2).to_broadcast([B8, H, WD])
                        nc.vector.tensor_mul(s[:], spsv, db)
                    s2 = small_pool.tile([B8, H, WD], f32, tag="s2")
                    nc.scalar.square(s2[:], s[:])
                    sq = small_pool.tile([B8, H], f32, tag="sq")
                    nc.vector.reduce_sum(sq[:], s2[:], axis=AX.X)
                    t0 = small_pool.tile([B8, H], f32, tag="t0")
                    nc.scalar.sqrt(t0[:], sq[:])
                    # one Newton step: t1 = 0.5*(t0 + sq/t0)
                    rt = small_pool.tile([B8, H], f32, tag="rt")
                    nc.vector.reciprocal(rt[:], t0[:])
                    tm = small_pool.tile([B8, H], f32, tag="tm")
                    nc.vector.tensor_mul(tm[:], sq[:], rt[:])
                    nc.vector.tensor_add(tm[:], tm[:], t0[:])
                    # r = 1/(1+sq); f = 0.5*t1*r folded: f = tm * 0.5 * r
                    onep = small_pool.tile([B8, H], f32, tag="onep")
                    nc.vector.tensor_scalar_add(onep[:], sq[:], 1.0)
                    rr = small_pool.tile([B8, H], f32, tag="rr")
                    nc.vector.reciprocal(rr[:], onep[:])
                    f = small_pool.tile([B8, H], f32, tag="f")
                    nc.vector.tensor_mul(f[:], tm[:], rr[:])
                    nc.vector.tensor_scalar_mul(f[:], f[:], 0.5)
                    v = small_pool.tile([B8, H, WD], f32, tag="v")
                    fb = f[:].unsqueeze(2).to_broadcast([B8, H, WD])
                    nc.vector.tensor_mul(v[:], s[:], fb)

                    if it == 2:
                        nc.sync.dma_start(
                            out_d[g * B8 : (g + 1) * B8, :],
                            v[:].rearrange("b h w -> b (h w)"),
                        )
                    else:
                        # broadcast v across partitions via PE, then agreement
                        vb = psum_r.tile([P, HW], f32, tag="vb")
                        nc.tensor.matmul(
                            vb[:],
                            srepl[:],
                            v[:].rearrange("b h w -> b (h w)"),
                            start=True,
                            stop=True,
                        )
                        for q in range(NQ):
                            pr = scr_pool.tile([P, ICC, H, WD], f32, tag="pr")
                            vbb = (
                                vb[:]
                                .rearrange("p (h w) -> p h w", h=H)
                                .unsqueeze(1)
                                .to_broadcast([P, ICC, H, WD])
                            )
                            nc.vector.tensor_mul(
                                pr[:], u[:, q * ICC : (q + 1) * ICC], vbb
                            )
                            if it == 0:
                                nc.vector.reduce_sum(
                                    logits[:, q * ICC : (q + 1) * ICC],
                                    pr[:],
                                    axis=AX.X,
                                )
                            else:
                                at = small_pool.tile([P, ICC, H], f32, tag="at")
                                nc.vector.reduce_sum(at[:], pr[:], axis=AX.X)
                                nc.vector.tensor_add(
                                    logits[:, q * ICC : (q + 1) * ICC],
                                    logits[:, q * ICC : (q + 1) * ICC],
                                    at[:],
                                )

    nc.compile()
    return nc


def _host_inputs(x: np.ndarray, W: np.ndarray):
    """Build per-core input maps."""
    xr = np.ascontiguousarray(x.reshape(B_FULL, NI, S).astype(np.float32, copy=False))
    W0 = np.asarray(W, dtype=np.float32).reshape(H, NI, WD, S)
    # wpack[ic, (i16,s), (h,w)] = W0[h, ic*16+i16, w, s]
    wpack = np.ascontiguousarray(
        W0.reshape(H, IC, I16, WD, S).transpose(1, 2, 4, 0, 3).reshape(IC, P, HW)
    )
    # sdelta[p, b'] = (p % 8 == b');  srepl = sdelta.T
    pidx = np.arange(P)
    sdelta = (pidx[:, None] % B8 == np.arange(B8)[None, :]).astype(np.float32)
    srepl = np.ascontiguousarray(sdelta.T)

    in_maps = []
    for c in range(N_CORES):
        xc = xr[c * B_CORE : (c + 1) * B_CORE]  # [32, 1152, 8]
        # xdiag[g, (i16,s), ic*128 + i16*8 + b] = xc[g*8+b, ic*16+i16, s]
        xd = np.zeros((NGRP, P, IC, I16, B8), dtype=np.float32)
        xg = xc.reshape(NGRP, B8, IC, I16, S)  # (g, b, ic, i16, s)
        for k in range(I16):
            # dest partitions k*8..k*8+8 (s), free (ic, i16=k, b)
            xd[:, k * S : (k + 1) * S, :, k, :] = xg[:, :, :, k, :].transpose(
                0, 3, 2, 1
            )
        in_maps.append(
            {
                "xdiag": np.ascontiguousarray(xd.reshape(NGRP, P, IC * P)),
                "wpack": wpack,
                "sdelta": sdelta,
                "srepl": srepl,
            }
        )
    return in_maps


def kernel(x: np.ndarray, W: np.ndarray) -> np.ndarray:
    from concourse import bass_utils

    if "nc" not in _CACHE:
        _CACHE["nc"] = _build_program(debug=False)
    nc = _CACHE["nc"]
    in_maps = _host_inputs(x, W)
    res = bass_utils.run_bass_kernel_spmd(nc, in_maps, list(range(N_CORES)))
    outs = [res.results[c]["vout"].reshape(B_CORE, H, WD) for c in range(N_CORES)]
    return np.concatenate(outs, axis=0).astype(np.float32)
